# revision 28
# baseline (speedup 1.0000x reference)
"""Trainium2 Bass kernel for nn_LocalContrastiveLoss.

Strategy (data-parallel over B, 1 image per core, 8 cores):
  - Host re-lays-out inputs per image (layout only, no arithmetic beyond
    dtype casts):
      * embeddings [E=64, HW=65536] -> transposed fp8e3 (e3m4) tiles so that
        pixel chunks of 128 land on SBUF partitions: [NG=8, 128, NCG=64 * 64]
        (fp8 halves HBM traffic vs bf16; e3m4 keeps 4 mantissa bits and the
        class means average ~8K pixels so the quantization noise washes out)
      * masks_onehot -> chunk-transposed one-hot planes [128, K, 512] fp8
      * aux = [z | sel]: z = sampled pixel embeddings [32,64] f32 (gather),
        sel = one-hot [32, 8] selecting each sample's own class column
  - Device per core:
      * 512 accumulating matmuls: lhsT=planes chunk [128,8] fp8, rhs=embT
        chunk [128,64] fp8 -> PSUM [8,64] = per-class embedding sums over all
        pixels (division by counts cancels under cosine normalization);
        4 consecutive chunks packed into the 4 PE column-groups
      * normalize class means and z rows with rsqrt(s) = exp(-0.5*ln(s)) so
        the whole kernel uses the single natural_log_exp_and_others ACT
        table set (preloaded up front -> no table switches in the tail)
      * sims = znT.T @ mnT -> [32, 8]; logsumexp over classes (no max-shift:
        |sims| <= 1/TEMP = 5); s_pos via sel mask
      * DMA the 32 per-term losses out; host sums across terms and cores
"""

import numpy as np
import ml_dtypes

import concourse.bass as bass
import concourse.bacc as bacc
import concourse.tile as tile
from concourse import mybir
from concourse.bass_utils import run_bass_kernel_spmd
from concourse.hw_specs import get_activation_tables
from concourse.masks import make_identity

B, E, H, W, K, NPOS = 8, 64, 256, 256, 8, 4
HW = H * W
TEMP = 0.2
NCHUNK = HW // 128          # 512 chunks of 128 pixels
NCG = 64                    # chunks per DMA group
NG = NCHUNK // NCG          # 8 groups (512 KiB fp8 per DMA)
NJ = K * NPOS               # 32 sampled pixels per image

f32 = mybir.dt.float32
fp8 = mybir.dt.float8e3     # e3m4
np_fp8 = ml_dtypes.float8_e3m4

Act = mybir.ActivationFunctionType


def build_bass():
    nc = bacc.Bacc(None, target_bir_lowering=False)

    embT = nc.dram_tensor("embT", [NG, 128, NCG * E], fp8, kind="ExternalInput")
    # planes split in two halves so the first matmuls are gated by a 256 KiB
    # transfer instead of 512 KiB (both halves ride the fast Sync ring)
    planes_in = nc.dram_tensor(
        "planesT", [2, 128, K * (NCHUNK // 2)], fp8, kind="ExternalInput"
    )
    aux_in = nc.dram_tensor("aux", [NJ, E + K], f32, kind="ExternalInput")
    out = nc.dram_tensor("out", [NJ, 1], f32, kind="ExternalOutput")

    with tile.TileContext(nc) as tc:
        with (
            tc.tile_pool(name="big", bufs=NG) as big,
            tc.tile_pool(name="small", bufs=1) as small,
            tc.tile_pool(name="psum", bufs=1, space="PSUM") as psum,
        ):
            # Preload the one ACT table set the whole kernel uses
            # (ln, exp, square all live in natural_log_exp_and_others).
            tables = list(get_activation_tables(nc.m.arch))
            set_id = tables.index("natural_log_exp_and_others")
            nc.scalar.add_instruction(
                mybir.InstLoadActFuncSet(
                    name=nc.get_next_instruction_name(),
                    act_func_set_id=set_id,
                    ins=[],
                    outs=[],
                )
            )

            # --- input DMAs; aux rides the (slow-for-bulk) ACT HWDGE ring,
            # all bulk transfers ride the Sync ring. The first matmuls are
            # gated by ph0 (256 KiB) + a small 16-chunk embT slice (64 KiB)
            # instead of planes+group0 (1 MiB serialized).
            aux = small.tile([NJ, E + K], f32)
            nc.scalar.dma_start(out=aux, in_=aux_in[:, :])
            ph0 = small.tile([128, K, NCHUNK // 2], fp8)
            nc.sync.dma_start(out=ph0, in_=planes_in[0, :, :])
            ph1 = small.tile([128, K, NCHUNK // 2], fp8)
            z_sb = aux[:, 0:E]
            sel_sb = aux[:, E:E + K]

            ident = small.tile([NJ, NJ], f32)
            make_identity(nc, ident)

            # --- z path (independent of the means matmuls; overlaps them)
            zsq = small.tile([NJ, E], f32)
            znrm2 = small.tile([NJ, 1], f32)
            nc.scalar.activation(zsq, z_sb, Act.Square, accum_out=znrm2)
            zln = small.tile([NJ, 1], f32)
            nc.scalar.activation(zln, znrm2, Act.Ln)
            zr = small.tile([NJ, 1], f32)
            nc.scalar.activation(zr, zln, Act.Exp, scale=-0.5)  # 1/|z|
            zn = small.tile([NJ, E], f32)
            nc.vector.tensor_scalar(
                out=zn,
                in0=z_sb,
                scalar1=zr,
                scalar2=1.0 / TEMP,
                op0=mybir.AluOpType.mult,
                op1=mybir.AluOpType.mult,
            )

            # --- 512 accumulating matmuls: class sums [K, E].
            # Pack 4 consecutive chunks into the 4 PE column-groups
            # (tile_position) so they execute concurrently; each group
            # accumulates into its own 32-partition PSUM slice.
            SPLIT = 16  # chunks in the small stream-starting slice of group 0
            means_ps = psum.tile([128, E], f32)
            for g in range(NG):
                if g == 0:
                    et = big.tile([128, NCG * E], fp8)
                    nc.sync.dma_start(
                        out=et[:, 0:SPLIT * E], in_=embT[0, :, 0:SPLIT * E]
                    )
                    nc.sync.dma_start(
                        out=et[:, SPLIT * E:], in_=embT[0, :, SPLIT * E:]
                    )
                else:
                    et = big.tile([128, NCG * E], fp8)
                    nc.sync.dma_start(out=et, in_=embT[g, :, :])
                if g == 1:
                    # second planes half: needed from chunk 256, issued after
                    # the stream-critical transfers
                    nc.sync.dma_start(out=ph1, in_=planes_in[1, :, :])
                for cl in range(NCG):
                    c = g * NCG + cl
                    j = c % 4
                    ph = ph0 if c < NCHUNK // 2 else ph1
                    nc.tensor.matmul(
                        means_ps[32 * j:32 * j + K, :],
                        ph[:, :, c % (NCHUNK // 2)],
                        et[:, cl * E:(cl + 1) * E],
                        start=(c < 4),
                        stop=(c >= NCHUNK - 4),
                        tile_position=(0, 32 * j),
                        # the 4 col-group accumulators share a PSUM bank;
                        # the sim's zero-region check is coarser than HW
                        skip_group_check=True,
                    )

            # --- tail: merge accumulators, normalize means, sims, lse
            # (dual-PSUM operands are rejected by the BIR verifier, so
            # accumulate into SBUF with one PSUM operand per op; emitted
            # before the znT copy so the DVE queue head does not block on
            # the PE transpose that only finishes after the matmul stream)
            m_sb = small.tile([K, E], f32)
            nc.vector.tensor_copy(m_sb, means_ps[0:K, :])
            nc.vector.tensor_add(m_sb, m_sb, means_ps[32:32 + K, :])
            nc.vector.tensor_add(m_sb, m_sb, means_ps[64:64 + K, :])
            nc.vector.tensor_add(m_sb, m_sb, means_ps[96:96 + K, :])

            # z transpose after the matmul loop so it does not sit ahead of
            # the 512 matmuls in the PE queue; it only gates the sims matmul.
            znT_ps = psum.tile([E, NJ], f32)
            nc.tensor.transpose(znT_ps, zn, ident[:, :])
            znT = small.tile([E, NJ], f32)
            nc.vector.tensor_copy(znT, znT_ps)

            msq = small.tile([K, E], f32)
            mnrm2 = small.tile([K, 1], f32)
            nc.scalar.activation(msq, m_sb, Act.Square, accum_out=mnrm2)
            mln = small.tile([K, 1], f32)
            nc.scalar.activation(mln, mnrm2, Act.Ln)
            mr = small.tile([K, 1], f32)
            nc.scalar.activation(mr, mln, Act.Exp, scale=-0.5)  # 1/|m|
            mn = small.tile([K, E], f32)
            nc.vector.tensor_scalar_mul(mn, m_sb, mr)

            mnT_ps = psum.tile([E, K], f32)
            nc.tensor.transpose(mnT_ps, mn, ident[:K, :K])
            mnT = small.tile([E, K], f32)
            nc.vector.tensor_copy(mnT, mnT_ps)

            # sims[j, k] = zn[j] . mn[k]  (already scaled by 1/TEMP)
            sims_ps = psum.tile([NJ, K], f32)
            nc.tensor.matmul(sims_ps, znT, mnT, start=True, stop=True)

            # logsumexp over classes, no max-shift (|sims| <= 1/TEMP = 5)
            ex = small.tile([NJ, K], f32)
            sm = small.tile([NJ, 1], f32)
            nc.scalar.activation(ex, sims_ps, Act.Exp, accum_out=sm)
            den = small.tile([NJ, 1], f32)
            nc.scalar.activation(den, sm, Act.Ln)

            # s_pos = sum_k sims * sel  (fused multiply+row-sum)
            spt = small.tile([NJ, K], f32)
            sp = small.tile([NJ, 1], f32)
            nc.vector.scalar_tensor_tensor(
                out=spt,
                in0=sims_ps,
                scalar=1.0,
                in1=sel_sb,
                op0=mybir.AluOpType.mult,
                op1=mybir.AluOpType.mult,
                accum_out=sp,
            )

            # loss_j = den - sp ; host sums the 32 terms per core
            loss = small.tile([NJ, 1], f32)
            nc.vector.tensor_tensor(
                out=loss, in0=den, in1=sp, op=mybir.AluOpType.subtract
            )
            nc.sync.dma_start(out=out[:, :], in_=loss)

    if not nc.is_finalized():
        nc.finalize()
    return nc


def _prep_inputs(embeddings, masks_onehot, pos_pix):
    embf = np.ascontiguousarray(
        np.asarray(embeddings, dtype=np.float32).reshape(B, E, HW)
    )

    # embT grouped: [B, NG, 128, NCG*E] fp8, partition = pixel-within-chunk
    embT = embf.transpose(0, 2, 1).reshape(B, NG, NCG, 128, E)
    embT = np.ascontiguousarray(embT.transpose(0, 1, 3, 2, 4)).reshape(
        B, NG, 128, NCG * E
    ).astype(np_fp8)

    # planesT: chunk-transposed one-hot masks, split in chunk halves:
    # [B, 2, 128, K, NCHUNK//2] fp8
    m = np.asarray(masks_onehot, dtype=np.float32).reshape(
        B, K, 2, NCHUNK // 2, 128
    )
    planesT = np.ascontiguousarray(m.transpose(0, 2, 4, 1, 3)).reshape(
        B, 2, 128, K * (NCHUNK // 2)
    ).astype(np_fp8)

    # z gather (host): [B, NJ, E] f32, packed with sel into aux
    pix = np.asarray(pos_pix).reshape(B, NJ)
    z = np.stack([embf[b][:, pix[b]].T for b in range(B)]).astype(np.float32)
    sel = np.zeros((NJ, K), dtype=np.float32)
    sel[np.arange(NJ), np.arange(NJ) // NPOS] = 1.0
    aux = np.concatenate(
        [z, np.broadcast_to(sel, (B, NJ, K))], axis=2
    ).astype(np.float32)

    return [
        {
            "embT": np.ascontiguousarray(embT[b]),
            "planesT": np.ascontiguousarray(planesT[b]),
            "aux": np.ascontiguousarray(aux[b]),
        }
        for b in range(B)
    ]


def _run(embeddings, masks_onehot, pos_pix, trace=False):
    in_maps = _prep_inputs(embeddings, masks_onehot, pos_pix)
    nc = build_bass()
    res = run_bass_kernel_spmd(nc, in_maps, core_ids=list(range(B)), trace=trace)
    partials = [
        np.asarray(r["out"], dtype=np.float64).sum() for r in res.results
    ]
    total = sum(partials) / float(B * K * NPOS)
    return np.float32(total), res


def kernel(embeddings, masks_onehot, pos_pix):
    val, _ = _run(embeddings, masks_onehot, pos_pix)
    return np.asarray(val, dtype=np.float32)


# revision 29
# speedup vs baseline: 1.0881x; 1.0881x over previous
"""Trainium2 Bass kernel for nn_LocalContrastiveLoss.

Strategy (data-parallel over B, 1 image per core, 8 cores):
  - Host re-lays-out inputs per image (layout only, no arithmetic beyond
    dtype casts):
      * embeddings [E=64, HW=65536] -> transposed fp8e3 (e3m4) tiles so that
        pixel chunks of 128 land on SBUF partitions: [NG=8, 128, NCG=64 * 64]
        (fp8 halves HBM traffic vs bf16; e3m4 keeps 4 mantissa bits and the
        class means average ~8K pixels so the quantization noise washes out)
      * masks_onehot -> chunk-transposed one-hot planes [128, K, 512] fp8
      * aux = [z | sel]: z = sampled pixel embeddings [32,64] f32 (gather),
        sel = one-hot [32, 8] selecting each sample's own class column
  - Device per core:
      * 512 accumulating matmuls: lhsT=planes chunk [128,8] fp8, rhs=embT
        chunk [128,64] fp8 -> PSUM [8,64] = per-class embedding sums over all
        pixels (division by counts cancels under cosine normalization);
        4 consecutive chunks packed into the 4 PE column-groups
      * normalize class means and z rows with rsqrt(s) = exp(-0.5*ln(s)) so
        the whole kernel uses the single natural_log_exp_and_others ACT
        table set (preloaded up front -> no table switches in the tail)
      * sims = znT.T @ mnT -> [32, 8]; logsumexp over classes (no max-shift:
        |sims| <= 1/TEMP = 5); s_pos via sel mask
      * DMA the 32 per-term losses out; host sums across terms and cores
"""

import numpy as np
import ml_dtypes

import concourse.bass as bass
import concourse.bacc as bacc
import concourse.tile as tile
from concourse import mybir
from concourse.bass_utils import run_bass_kernel_spmd
from concourse.hw_specs import get_activation_tables
from concourse.masks import make_identity

B, E, H, W, K, NPOS = 8, 64, 256, 256, 8, 4
HW = H * W
TEMP = 0.2
NCHUNK = HW // 128          # 512 chunks of 128 pixels
NCG = 64                    # chunks per DMA group
NG = NCHUNK // NCG          # 8 groups (512 KiB fp8 per DMA)
NJ = K * NPOS               # 32 sampled pixels per image

f32 = mybir.dt.float32
fp8 = mybir.dt.float8e3     # e3m4
np_fp8 = ml_dtypes.float8_e3m4

Act = mybir.ActivationFunctionType


def build_bass():
    nc = bacc.Bacc(None, target_bir_lowering=False)

    embT = nc.dram_tensor("embT", [NG, 128, NCG * E], fp8, kind="ExternalInput")
    planes_in = nc.dram_tensor(
        "planesT", [128, K * NCHUNK], fp8, kind="ExternalInput"
    )
    aux_in = nc.dram_tensor("aux", [NJ, E + K], f32, kind="ExternalInput")
    out = nc.dram_tensor("out", [NJ, 1], f32, kind="ExternalOutput")

    with tile.TileContext(nc) as tc:
        with (
            tc.tile_pool(name="big", bufs=NG) as big,
            tc.tile_pool(name="small", bufs=1) as small,
            tc.tile_pool(name="psum", bufs=1, space="PSUM") as psum,
        ):
            # Preload the one ACT table set the whole kernel uses
            # (ln, exp, square all live in natural_log_exp_and_others).
            tables = list(get_activation_tables(nc.m.arch))
            set_id = tables.index("natural_log_exp_and_others")
            nc.scalar.add_instruction(
                mybir.InstLoadActFuncSet(
                    name=nc.get_next_instruction_name(),
                    act_func_set_id=set_id,
                    ins=[],
                    outs=[],
                )
            )

            # --- input DMAs; split across the two HWDGE rings (Sync + ACT)
            # so descriptor generation (~0.7us per dma_start) pipelines
            aux = small.tile([NJ, E + K], f32)
            nc.scalar.dma_start(out=aux, in_=aux_in[:, :])
            planes = small.tile([128, K, NCHUNK], fp8)
            nc.sync.dma_start(out=planes, in_=planes_in[:, :])
            z_sb = aux[:, 0:E]
            sel_sb = aux[:, E:E + K]

            ident = small.tile([NJ, NJ], f32)
            make_identity(nc, ident)

            # --- z path (independent of the means matmuls; overlaps them)
            zsq = small.tile([NJ, E], f32)
            znrm2 = small.tile([NJ, 1], f32)
            nc.scalar.activation(zsq, z_sb, Act.Square, accum_out=znrm2)
            zln = small.tile([NJ, 1], f32)
            nc.scalar.activation(zln, znrm2, Act.Ln)
            zr = small.tile([NJ, 1], f32)
            nc.scalar.activation(zr, zln, Act.Exp, scale=-0.5)  # 1/|z|
            zn = small.tile([NJ, E], f32)
            nc.vector.tensor_scalar(
                out=zn,
                in0=z_sb,
                scalar1=zr,
                scalar2=1.0 / TEMP,
                op0=mybir.AluOpType.mult,
                op1=mybir.AluOpType.mult,
            )

            # --- 512 accumulating matmuls: class sums [K, E].
            # Pack 4 consecutive chunks into the 4 PE column-groups
            # (tile_position) so they execute concurrently; each group
            # accumulates into its own 32-partition PSUM slice.
            means_ps = psum.tile([128, E], f32)
            for g in range(NG):
                et = big.tile([128, NCG * E], fp8)
                nc.sync.dma_start(out=et, in_=embT[g, :, :])
                for cl in range(NCG):
                    c = g * NCG + cl
                    j = c % 4
                    nc.tensor.matmul(
                        means_ps[32 * j:32 * j + K, :],
                        planes[:, :, c],
                        et[:, cl * E:(cl + 1) * E],
                        start=(c < 4),
                        stop=(c >= NCHUNK - 4),
                        tile_position=(0, 32 * j),
                        # the 4 col-group accumulators share a PSUM bank;
                        # the sim's zero-region check is coarser than HW
                        skip_group_check=True,
                    )

            # --- tail: merge accumulators, normalize means, sims, lse
            # (dual-PSUM operands are rejected by the BIR verifier, so
            # accumulate into SBUF with one PSUM operand per op; emitted
            # before the znT copy so the DVE queue head does not block on
            # the PE transpose that only finishes after the matmul stream)
            m_sb = small.tile([K, E], f32)
            nc.vector.tensor_copy(m_sb, means_ps[0:K, :])
            nc.vector.tensor_add(m_sb, m_sb, means_ps[32:32 + K, :])
            nc.vector.tensor_add(m_sb, m_sb, means_ps[64:64 + K, :])
            nc.vector.tensor_add(m_sb, m_sb, means_ps[96:96 + K, :])

            # z transpose after the matmul loop so it does not sit ahead of
            # the 512 matmuls in the PE queue; it only gates the sims matmul.
            znT_ps = psum.tile([E, NJ], f32)
            nc.tensor.transpose(znT_ps, zn, ident[:, :])
            znT = small.tile([E, NJ], f32)
            nc.vector.tensor_copy(znT, znT_ps)

            msq = small.tile([K, E], f32)
            mnrm2 = small.tile([K, 1], f32)
            nc.scalar.activation(msq, m_sb, Act.Square, accum_out=mnrm2)
            mln = small.tile([K, 1], f32)
            nc.scalar.activation(mln, mnrm2, Act.Ln)
            mr = small.tile([K, 1], f32)
            nc.scalar.activation(mr, mln, Act.Exp, scale=-0.5)  # 1/|m|
            mn = small.tile([K, E], f32)
            nc.vector.tensor_scalar_mul(mn, m_sb, mr)

            mnT_ps = psum.tile([E, K], f32)
            nc.tensor.transpose(mnT_ps, mn, ident[:K, :K])
            mnT = small.tile([E, K], f32)
            nc.vector.tensor_copy(mnT, mnT_ps)

            # sims[j, k] = zn[j] . mn[k]  (already scaled by 1/TEMP)
            sims_ps = psum.tile([NJ, K], f32)
            nc.tensor.matmul(sims_ps, znT, mnT, start=True, stop=True)

            # logsumexp over classes, no max-shift (|sims| <= 1/TEMP = 5)
            ex = small.tile([NJ, K], f32)
            sm = small.tile([NJ, 1], f32)
            nc.scalar.activation(ex, sims_ps, Act.Exp, accum_out=sm)
            den = small.tile([NJ, 1], f32)
            nc.scalar.activation(den, sm, Act.Ln)

            # s_pos = sum_k sims * sel  (fused multiply+row-sum)
            spt = small.tile([NJ, K], f32)
            sp = small.tile([NJ, 1], f32)
            nc.vector.scalar_tensor_tensor(
                out=spt,
                in0=sims_ps,
                scalar=1.0,
                in1=sel_sb,
                op0=mybir.AluOpType.mult,
                op1=mybir.AluOpType.mult,
                accum_out=sp,
            )

            # loss_j = den - sp ; host sums the 32 terms per core
            loss = small.tile([NJ, 1], f32)
            nc.vector.tensor_tensor(
                out=loss, in0=den, in1=sp, op=mybir.AluOpType.subtract
            )
            nc.sync.dma_start(out=out[:, :], in_=loss)

    if not nc.is_finalized():
        nc.finalize()
    return nc


def _prep_inputs(embeddings, masks_onehot, pos_pix):
    embf = np.ascontiguousarray(
        np.asarray(embeddings, dtype=np.float32).reshape(B, E, HW)
    )

    # embT grouped: [B, NG, 128, NCG*E] fp8, partition = pixel-within-chunk
    embT = embf.transpose(0, 2, 1).reshape(B, NG, NCG, 128, E)
    embT = np.ascontiguousarray(embT.transpose(0, 1, 3, 2, 4)).reshape(
        B, NG, 128, NCG * E
    ).astype(np_fp8)

    # planesT: chunk-transposed one-hot masks [B, 128, K, NCHUNK] fp8
    m = np.asarray(masks_onehot, dtype=np.float32).reshape(B, K, NCHUNK, 128)
    planesT = np.ascontiguousarray(m.transpose(0, 3, 1, 2)).reshape(
        B, 128, K * NCHUNK
    ).astype(np_fp8)

    # z gather (host): [B, NJ, E] f32, packed with sel into aux
    pix = np.asarray(pos_pix).reshape(B, NJ)
    z = np.stack([embf[b][:, pix[b]].T for b in range(B)]).astype(np.float32)
    sel = np.zeros((NJ, K), dtype=np.float32)
    sel[np.arange(NJ), np.arange(NJ) // NPOS] = 1.0
    aux = np.concatenate(
        [z, np.broadcast_to(sel, (B, NJ, K))], axis=2
    ).astype(np.float32)

    return [
        {
            "embT": np.ascontiguousarray(embT[b]),
            "planesT": np.ascontiguousarray(planesT[b]),
            "aux": np.ascontiguousarray(aux[b]),
        }
        for b in range(B)
    ]


def _run(embeddings, masks_onehot, pos_pix, trace=False):
    in_maps = _prep_inputs(embeddings, masks_onehot, pos_pix)
    nc = build_bass()
    res = run_bass_kernel_spmd(nc, in_maps, core_ids=list(range(B)), trace=trace)
    partials = [
        np.asarray(r["out"], dtype=np.float64).sum() for r in res.results
    ]
    total = sum(partials) / float(B * K * NPOS)
    return np.float32(total), res


def kernel(embeddings, masks_onehot, pos_pix):
    val, _ = _run(embeddings, masks_onehot, pos_pix)
    return np.asarray(val, dtype=np.float32)


# revision 32
# speedup vs baseline: 1.1265x; 1.0353x over previous
"""Trainium2 Bass kernel for nn_LocalContrastiveLoss.

Strategy (data-parallel over B, 1 image per core, 8 cores):
  - Host re-lays-out inputs per image (layout only, no arithmetic beyond
    dtype casts):
      * embeddings [E=64, HW=65536] -> transposed fp8e3 (e3m4) tiles so that
        pixel chunks of 128 land on SBUF partitions: [NG=8, 128, NCG=64 * 64]
        (fp8 halves HBM traffic vs bf16; e3m4 keeps 4 mantissa bits and the
        class means average ~8K pixels so the quantization noise washes out)
      * masks_onehot -> chunk-transposed one-hot planes [128, K, 512] fp8
      * aux = [z | sel]: z = sampled pixel embeddings [32,64] f32 (gather),
        sel = one-hot [32, 8] selecting each sample's own class column
  - Device per core:
      * 512 accumulating matmuls: lhsT=planes chunk [128,8] fp8, rhs=embT
        chunk [128,64] fp8 -> PSUM [8,64] = per-class embedding sums over all
        pixels (division by counts cancels under cosine normalization);
        4 consecutive chunks packed into the 4 PE column-groups
      * normalize class means and z rows with rsqrt(s) = exp(-0.5*ln(s)) so
        the whole kernel uses the single natural_log_exp_and_others ACT
        table set (preloaded up front -> no table switches in the tail)
      * sims = znT.T @ mnT -> [32, 8]; logsumexp over classes (no max-shift:
        |sims| <= 1/TEMP = 5); s_pos via sel mask
      * DMA the 32 per-term losses out; host sums across terms and cores
"""

import numpy as np
import ml_dtypes

import concourse.bass as bass
import concourse.bacc as bacc
import concourse.tile as tile
from concourse import mybir
from concourse.bass_utils import run_bass_kernel_spmd
from concourse.hw_specs import get_activation_tables
from concourse.masks import make_identity

B, E, H, W, K, NPOS = 8, 64, 256, 256, 8, 4
HW = H * W
TEMP = 0.2
NCHUNK = HW // 128          # 512 chunks of 128 pixels
NCG = 64                    # chunks per DMA group
NG = NCHUNK // NCG          # 8 groups (512 KiB fp8 per DMA)
NJ = K * NPOS               # 32 sampled pixels per image

f32 = mybir.dt.float32
fp8 = mybir.dt.float8e3     # e3m4
np_fp8 = ml_dtypes.float8_e3m4

Act = mybir.ActivationFunctionType


def build_bass():
    nc = bacc.Bacc(None, target_bir_lowering=False)

    embT = nc.dram_tensor("embT", [NG, 128, NCG * E], fp8, kind="ExternalInput")
    planes_in = nc.dram_tensor(
        "planesT", [128, K * NCHUNK], fp8, kind="ExternalInput"
    )
    aux_in = nc.dram_tensor("aux", [NJ, E + K], f32, kind="ExternalInput")
    out = nc.dram_tensor("out", [NJ, 1], f32, kind="ExternalOutput")

    with tile.TileContext(nc) as tc:
        with (
            tc.tile_pool(name="big", bufs=NG) as big,
            tc.tile_pool(name="small", bufs=1) as small,
            tc.tile_pool(name="psum", bufs=1, space="PSUM") as psum,
        ):
            # Preload the one ACT table set the whole kernel uses
            # (ln, exp, square all live in natural_log_exp_and_others).
            tables = list(get_activation_tables(nc.m.arch))
            set_id = tables.index("natural_log_exp_and_others")
            nc.scalar.add_instruction(
                mybir.InstLoadActFuncSet(
                    name=nc.get_next_instruction_name(),
                    act_func_set_id=set_id,
                    ins=[],
                    outs=[],
                )
            )

            # --- input DMAs; split across the two HWDGE rings (Sync + ACT)
            # so descriptor generation (~0.7us per dma_start) pipelines
            aux = small.tile([NJ, E + K], f32)
            nc.scalar.dma_start(out=aux, in_=aux_in[:, :])
            planes = small.tile([128, K, NCHUNK], fp8)
            nc.sync.dma_start(out=planes, in_=planes_in[:, :])
            z_sb = aux[:, 0:E]
            sel_sb = aux[:, E:E + K]

            ident = small.tile([NJ, NJ], f32)
            make_identity(nc, ident)

            # --- z path (independent of the means matmuls; overlaps them)
            zsq = small.tile([NJ, E], f32)
            znrm2 = small.tile([NJ, 1], f32)
            nc.scalar.activation(zsq, z_sb, Act.Square, accum_out=znrm2)
            zln = small.tile([NJ, 1], f32)
            nc.scalar.activation(zln, znrm2, Act.Ln)
            zr = small.tile([NJ, 1], f32)
            nc.scalar.activation(zr, zln, Act.Exp, scale=-0.5)  # 1/|z|
            zn = small.tile([NJ, E], f32)
            nc.vector.tensor_scalar(
                out=zn,
                in0=z_sb,
                scalar1=zr,
                scalar2=1.0 / TEMP,
                op0=mybir.AluOpType.mult,
                op1=mybir.AluOpType.mult,
            )

            # --- 512 accumulating matmuls: class sums [K, E].
            # Pack 4 consecutive chunks into the 4 PE column-groups
            # (tile_position) so they execute concurrently; each group
            # accumulates into its own 32-partition PSUM slice.
            means_ps = psum.tile([128, E], f32)
            for g in range(NG):
                et = big.tile([128, NCG * E], fp8)
                nc.sync.dma_start(out=et, in_=embT[g, :, :])
                for cl in range(NCG):
                    c = g * NCG + cl
                    j = c % 4
                    nc.tensor.matmul(
                        means_ps[32 * j:32 * j + K, :],
                        planes[:, :, c],
                        et[:, cl * E:(cl + 1) * E],
                        start=(c < 4),
                        stop=(c >= NCHUNK - 4),
                        tile_position=(0, 32 * j),
                        # the 4 col-group accumulators share a PSUM bank;
                        # the sim's zero-region check is coarser than HW
                        skip_group_check=True,
                    )

            # --- tail: merge accumulators, normalize means, sims, lse
            # (dual-PSUM operands are rejected by the BIR verifier, so
            # accumulate into SBUF with one PSUM operand per op; emitted
            # before the znT copy so the DVE queue head does not block on
            # the PE transpose that only finishes after the matmul stream)
            m_sb = small.tile([K, E], f32)
            nc.vector.tensor_copy(m_sb, means_ps[0:K, :])
            nc.vector.tensor_add(m_sb, m_sb, means_ps[32:32 + K, :])
            nc.vector.tensor_add(m_sb, m_sb, means_ps[64:64 + K, :])
            nc.vector.tensor_add(m_sb, m_sb, means_ps[96:96 + K, :])

            # z transpose after the matmul loop so it does not sit ahead of
            # the 512 matmuls in the PE queue; it only gates the sims matmul.
            znT_ps = psum.tile([E, NJ], f32)
            nc.tensor.transpose(znT_ps, zn, ident[:, :])
            znT = small.tile([E, NJ], f32)
            nc.vector.tensor_copy(znT, znT_ps)

            msq = small.tile([K, E], f32)
            mnrm2 = small.tile([K, 1], f32)
            nc.scalar.activation(msq, m_sb, Act.Square, accum_out=mnrm2)
            mln = small.tile([K, 1], f32)
            nc.scalar.activation(mln, mnrm2, Act.Ln)
            mr = small.tile([K, 1], f32)
            nc.scalar.activation(mr, mln, Act.Exp, scale=-0.5)  # 1/|m|
            mn = small.tile([K, E], f32)
            nc.vector.tensor_scalar_mul(mn, m_sb, mr)

            mnT_ps = psum.tile([E, K], f32)
            nc.tensor.transpose(mnT_ps, mn, ident[:K, :K])
            mnT = small.tile([E, K], f32)
            nc.vector.tensor_copy(mnT, mnT_ps)

            # sims[j, k] = zn[j] . mn[k]  (already scaled by 1/TEMP)
            sims_ps = psum.tile([NJ, K], f32)
            nc.tensor.matmul(sims_ps, znT, mnT, start=True, stop=True)

            # logsumexp over classes, no max-shift (|sims| <= 1/TEMP = 5)
            ex = small.tile([NJ, K], f32)
            sm = small.tile([NJ, 1], f32)
            nc.scalar.activation(ex, sims_ps, Act.Exp, accum_out=sm)
            den = small.tile([NJ, 1], f32)
            nc.scalar.activation(den, sm, Act.Ln)

            # s_pos = sum_k sims * sel  (fused multiply+row-sum)
            spt = small.tile([NJ, K], f32)
            sp = small.tile([NJ, 1], f32)
            nc.vector.scalar_tensor_tensor(
                out=spt,
                in0=sims_ps,
                scalar=1.0,
                in1=sel_sb,
                op0=mybir.AluOpType.mult,
                op1=mybir.AluOpType.mult,
                accum_out=sp,
            )

            # loss_j = den - sp ; host sums the 32 terms per core
            loss = small.tile([NJ, 1], f32)
            nc.vector.tensor_tensor(
                out=loss, in0=den, in1=sp, op=mybir.AluOpType.subtract
            )
            nc.sync.dma_start(out=out[:, :], in_=loss)

    if not nc.is_finalized():
        nc.finalize()
    return nc


def _prep_inputs(embeddings, masks_onehot, pos_pix):
    embf = np.ascontiguousarray(
        np.asarray(embeddings, dtype=np.float32).reshape(B, E, HW)
    )

    # embT grouped: [B, NG, 128, NCG*E] fp8, partition = pixel-within-chunk
    embT = embf.transpose(0, 2, 1).reshape(B, NG, NCG, 128, E)
    embT = np.ascontiguousarray(embT.transpose(0, 1, 3, 2, 4)).reshape(
        B, NG, 128, NCG * E
    ).astype(np_fp8)

    # planesT: chunk-transposed one-hot masks [B, 128, K, NCHUNK] fp8
    m = np.asarray(masks_onehot, dtype=np.float32).reshape(B, K, NCHUNK, 128)
    planesT = np.ascontiguousarray(m.transpose(0, 3, 1, 2)).reshape(
        B, 128, K * NCHUNK
    ).astype(np_fp8)

    # z gather (host): [B, NJ, E] f32, packed with sel into aux
    pix = np.asarray(pos_pix).reshape(B, NJ)
    z = np.stack([embf[b][:, pix[b]].T for b in range(B)]).astype(np.float32)
    sel = np.zeros((NJ, K), dtype=np.float32)
    sel[np.arange(NJ), np.arange(NJ) // NPOS] = 1.0
    aux = np.concatenate(
        [z, np.broadcast_to(sel, (B, NJ, K))], axis=2
    ).astype(np.float32)

    return [
        {
            "embT": np.ascontiguousarray(embT[b]),
            "planesT": np.ascontiguousarray(planesT[b]),
            "aux": np.ascontiguousarray(aux[b]),
        }
        for b in range(B)
    ]


def _run(embeddings, masks_onehot, pos_pix, trace=False):
    in_maps = _prep_inputs(embeddings, masks_onehot, pos_pix)
    nc = build_bass()
    res = run_bass_kernel_spmd(nc, in_maps, core_ids=list(range(B)), trace=trace)
    partials = [
        np.asarray(r["out"], dtype=np.float64).sum() for r in res.results
    ]
    total = sum(partials) / float(B * K * NPOS)
    return np.float32(total), res


def kernel(embeddings, masks_onehot, pos_pix):
    val, _ = _run(embeddings, masks_onehot, pos_pix)
    return np.asarray(val, dtype=np.float32)
